# revision 50
# baseline (speedup 1.0000x reference)
"""Trainium2 Bass kernel: multi-head attention block (dense transformer).

Reference computation (fp32):
    qkv = x @ w_qkv.T            x:[4,2048,1024]  w_qkv:[3072,1024]
    q,k,v per 16 heads (hd=64);  S = q@k.T * hd**-0.5; P = softmax(S)
    out = (P@v) heads-merged;    y = out @ w_proj.T + b_proj

Sharding (8 cores, no collectives): core = (batch b, token-half).  Each core
computes k/v for its whole batch (replicated across the 2 half-cores) and
q / attention / proj for its own 1024 tokens, writing a disjoint
y[b, half] slice.

All matmul operands bf16 with fp32 PSUM accumulation (fp8 would be ~2x
faster on the PE but softmax weight noise passes straight through to the
output -- the positive-sum normalization shrinks signal and noise equally --
so per-element precision must stay at bf16 for the 2e-2 gate).

On-chip layout: feature-major ([d, t]), no activation transposes:
    kT,qT: [d, t] from matmul(lhsT=w.T tile, rhs=x.T tile)
    S.T [m, (e|o) n-chunk]: per chunk-iter one [128, 2, 512] PSUM tile, two
          matmuls (two heads side by side) so ONE ScalarE Exp covers 1024
          columns.
    P.T = Exp(S.T * scale) bf16 (max-subtraction unnecessary: |S*scale|<~7)
    v_aug [t, 65] per head: v with a ones column -> attn@v matmul yields
          out.T[0:64] AND the softmax denominators in row 64, accumulated
          over m in PSUM.
    normalize (deferred): reciprocal runs straight off the PSUM denom row,
          raw numerators copy to SBUF so the banks free immediately; the
          partition-0 DMA -> GpSimd broadcast -> multiply chain runs off the
          critical path, writing oat.
    yT = matmul(lhsT=w_proj.T, rhs=out_attn.T) + bias (DVE add)

Schedule: one flat software pipeline over (pair, col-chunk, m-tile) at
512-column granularity -- exp of chunk i issues first, attn@v lags AVLAG
behind, scores for chunk i+1 issue last.  Iterating m-tiles innermost makes
the attn@v accumulators single PSUM banks, so the score tag gets THREE
[128,1024] buffers: every buffer-rotation user (scores, k/q/v projection
fillers) is >= a full step away from its buffer-mate's reader and the PE
never idles on the score->exp ping-pong.  k/q/v projections for later pairs
are woven in as PE filler work; per-pair weights stream with 2-deep
prefetch, one DMA per tensor (host pre-arranges layouts for 2KB lines).
The output projection runs in two rounds of 8 groups; round 1 accumulates
pairs 0..6 while the last chunk's normalize chain completes.
"""

import os

os.environ.setdefault("MYCRO_LOCAL_CACHE", "1")

from contextlib import ExitStack

import ml_dtypes
import numpy as np

import concourse.tile as tile
from concourse import bacc, mybir
from concourse.bass_utils import run_bass_kernel_spmd

# Problem shape (hardcoded per contract)
B, N, C = 4, 2048, 1024
HEADS, HD = 16, 64
SCALE = HD**-0.5  # 0.125
TOWN = 1024  # q tokens owned per core
NCORES = 8
P = 128
CT = C // P  # 8 contraction tiles
MT = N // P  # 16 m (key-token) tiles
PAIRS = HEADS // 2  # 8 head pairs (2 heads share a 128-row tile)
KCH = N // 512  # 4 k-token chunks of 512
NCH = TOWN // 512  # 2 q-token chunks of 512

FP32 = mybir.dt.float32
BF16 = mybir.dt.bfloat16
EXP = mybir.ActivationFunctionType.Exp

_CACHE = {}


def _emit(tc, aps):
    nc = tc.nc
    xt, wqt, wkt, wvt, wpt, bias_d, yt = (
        aps["xt"], aps["wqt"], aps["wkt"], aps["wvt"], aps["wpt"],
        aps["bias"], aps["yt"],
    )

    ctx = ExitStack()
    const_pool = ctx.enter_context(tc.tile_pool(name="const", bufs=1))
    wpool = ctx.enter_context(tc.tile_pool(name="w", bufs=1))
    xpool = ctx.enter_context(tc.tile_pool(name="x", bufs=1))
    kqv = ctx.enter_context(tc.tile_pool(name="kqv", bufs=1))
    apool = ctx.enter_context(tc.tile_pool(name="attn", bufs=1))
    opool = ctx.enter_context(tc.tile_pool(name="oattn", bufs=1))
    ypool = ctx.enter_context(tc.tile_pool(name="y", bufs=1))
    psum = ctx.enter_context(tc.tile_pool(name="ps", bufs=1, space="PSUM"))

    bias_sb = const_pool.tile([P, 8], FP32, name="bias_sb")

    # x loads + per-pair weight slices.  Host layouts give every DMA >=1KB
    # contiguous lines and one dma_start per tensor slice:
    #   wqt/wkt: [PAIRS, P, C]    (partition-major: [part, ci*P+f])
    #   wvt:     [DUOS,  P, 2C]   (partition-major: [part, (ci, pp, f)])
    wp = [wpool.tile([P, C], BF16, name=f"wp{i}", tag=f"wp{i}") for i in range(CT)]
    xs = [xpool.tile([P, N], BF16, name=f"x{i}", tag=f"x{i}") for i in range(CT)]
    wpair = {}

    def load_pair_weights(p):
        for kind, src in (("k", wkt), ("q", wqt)):
            t = wpool.tile([P, C], BF16, tag=f"w{kind}p", bufs=2,
                           name=f"w{kind}p{p}")
            wpair[(kind, p)] = t
            nc.sync.dma_start(t[:], src[p])

    def load_duo_weights(duo):
        """v weights for a duo (pairs 2*duo, 2*duo+1): [128, CT x 256] tile."""
        t = wpool.tile([P, CT, 2 * P], BF16, tag="wvd", bufs=2, name=f"wvd{duo}")
        wpair[("v", duo)] = t
        nc.sync.dma_start(t[:], wvt[duo])

    # startup loads, ordered by first use
    wk0 = wpool.tile([P, C], BF16, tag="wkp", bufs=2, name="wkp0")
    wq0 = wpool.tile([P, C], BF16, tag="wqp", bufs=2, name="wqp0")
    wpair[("k", 0)], wpair[("q", 0)] = wk0, wq0
    rows = lambda i: slice(i * P, (i + 1) * P)
    nc.sync.dma_start(wk0[:], wkt[0])
    for i in range(CT):
        nc.sync.dma_start(xs[i][:, 0:512], xt[rows(i), 0:512])
    nc.sync.dma_start(wq0[:], wqt[0])
    load_duo_weights(0)
    load_pair_weights(1)
    # bulk of x on the scalar queue so in-loop weight prefetches (sync queue)
    # aren't stuck behind these large transfers
    for i in range(CT):
        nc.scalar.dma_start(xs[i][:, 512:2048], xt[rows(i), 512:2048])
    nc.sync.dma_start(bias_sb[:], bias_d[:])

    # persistent activations
    kt = [kqv.tile([P, N], BF16, name=f"kt{p}", tag=f"kt{p}") for p in range(CT)]
    qt = [kqv.tile([P, TOWN], BF16, name=f"qt{p}", tag=f"qt{p}") for p in range(CT)]
    # v_aug per pair: [128 tokens, 16 m-tiles, 2 heads, 65] bf16; col 64 = ones
    va = [kqv.tile([P, MT, 2, HD + 1], BF16, name=f"va{p}", tag=f"va{p}")
          for p in range(PAIRS)]
    for p in range(PAIRS):
        nc.vector.memset(va[p][:, :, :, HD : HD + 1], 1.0)
    oat = [opool.tile([P, TOWN], BF16, name=f"oat{p}", tag=f"oat{p}")
           for p in range(PAIRS)]

    def kq_group(p, kind, ch):
        """One 512-col chunk of the k or q projection for pair p."""
        w, dst = wpair[(kind, p)], (kt if kind == "k" else qt)
        ps = psum.tile([P, 512], FP32, tag="st", bufs=3, name="fill_st")
        cols = slice(ch * 512, (ch + 1) * 512)
        for ci in range(CT):
            nc.tensor.matmul(
                ps[:], w[:, ci * P : (ci + 1) * P], xs[ci][:, cols],
                start=(ci == 0), stop=(ci == CT - 1),
            )
        nc.vector.tensor_copy(dst[p][:, cols], ps[:])

    def v_group(duo, mt):
        """v for token tile mt, one duo = 2 pairs (256 d-cols), just-in-time."""
        w = wpair[("v", duo)]
        ps = psum.tile([P, 2 * P], FP32, tag="st", bufs=3, name="fill_st")
        for ci in range(CT):
            nc.tensor.matmul(
                ps[:], xs[ci][:, mt * P : (mt + 1) * P], w[:, ci, :],
                start=(ci == 0), stop=(ci == CT - 1),
            )
        for pp in range(2):
            nc.vector.tensor_copy(
                va[2 * duo + pp][:, mt, :, 0:HD],
                ps[:, pp * P : (pp + 1) * P].rearrange("t (h d) -> t h d", h=2),
            )

    # startup: only the k/q chunks the first score steps need (k cols 0:512
    # cover m-tiles 0..3, q chunk 0 covers the first 16 chunk-steps); the
    # remaining pair-0 projection chunks are woven in as early fillers
    kq_jobs = [("k", ch) for ch in range(KCH)] + [("q", ch) for ch in range(NCH)]
    kq_group(0, "k", 0)
    kq_group(0, "q", 0)

    # ---- attention pipeline over (pair, col-chunk, m-tile) ----
    av_cur = {}

    def st_chunk(p, ch, mt):
        """Scores for one 512-col q chunk, both heads side by side so one
        ScalarE Exp covers 1024 columns."""
        st = psum.tile([P, 2, 512], FP32, tag="st", bufs=3,
                       name=f"st{p}_{ch}_{mt}")
        ms = slice(mt * P, (mt + 1) * P)
        cs = slice(ch * 512, (ch + 1) * 512)
        for h in range(2):
            nc.tensor.matmul(st[:, h, :], kt[p][64 * h : 64 * h + 64, ms],
                             qt[p][64 * h : 64 * h + 64, cs],
                             start=True, stop=True)
        return st

    def exp_chunk(st):
        pt = apool.tile([P, 2, 512], BF16, tag="pt", bufs=9, name="pt")
        nc.scalar.activation(pt[:], st[:], EXP, scale=SCALE)
        return pt

    def av_chunk(p, ch, mt, pt):
        if mt == 0:
            av_cur["e"] = psum.tile([P, 512], FP32, tag="av_e",
                                    name=f"av_e{p}_{ch}")
            av_cur["o"] = psum.tile([P, 512], FP32, tag="av_o",
                                    name=f"av_o{p}_{ch}")
        nc.tensor.matmul(av_cur["e"][0:65, :], va[p][:, mt, 0, :], pt[:, 0, :],
                         start=(mt == 0), stop=(mt == MT - 1))
        nc.tensor.matmul(av_cur["o"][0:65, :], va[p][:, mt, 1, :], pt[:, 1, :],
                         start=(mt == 0), stop=(mt == MT - 1))

    def fillers(p, vmt):
        """Filler work at virtual step vmt = ch*MT + mt within pair p."""
        if vmt == 1 and 1 <= p < PAIRS - 1:
            load_pair_weights(p + 1)
        d = p // 2
        if p == 0:
            if vmt == 1:
                load_duo_weights(1)
            # rest of pair 0's own k/q projection, just ahead of first use
            if vmt in (1, 4, 8):
                kq_group(0, "k", 1 + (1, 4, 8).index(vmt))
            elif vmt == 12:
                kq_group(0, "q", 1)
            if vmt < MT:
                v_group(0, vmt)  # duo 0: one group per step, first pass
        elif p % 2 == 0:
            if vmt == 1 and d + 1 < PAIRS // 2:
                load_duo_weights(d + 1)
            if 2 <= vmt <= 16 and vmt % 2 == 0:
                v_group(d, 8 + (vmt - 2) // 2)  # tail half: tiles 8..15
        else:
            if d + 1 < PAIRS // 2 and 2 <= vmt <= 16 and vmt % 2 == 0:
                v_group(d + 1, (vmt - 2) // 2)  # head half of next duo
        if p < PAIRS - 1 and 17 <= vmt <= 27 and vmt % 2 == 1:
            kq_group(p + 1, *kq_jobs[(vmt - 17) // 2])
        if p == 3 and vmt == 0:
            # scalar queue: these large transfers must not delay the sync
            # queue's in-loop pair-weight prefetches
            for i in range(CT):
                nc.scalar.dma_start(wp[i][:], wpt[i * P : (i + 1) * P, :])

    def drain_av(p, ch):
        """Free the av PSUM banks: reciprocal straight off the denom rows
        (starts the p0-DMA -> broadcast chain early), then raw copies."""
        out = []
        for h, av_x in (("e", av_cur["e"]), ("o", av_cur["o"])):
            r = apool.tile([P, 512], BF16, tag="rcp", bufs=2, name=f"rcp_{h}{p}")
            with nc.allow_low_precision(reason="softmax denom recip"):
                nc.vector.reciprocal(r[64:65, :], av_x[64:65, :])
            nc.sync.dma_start(r[0:1, :], r[64:65, :])
            rb = apool.tile([P, 512], BF16, tag="rb", bufs=2, name=f"rb_{h}{p}")
            nc.gpsimd.partition_broadcast(rb[0:64, :], r[0:1, :], channels=64)
            raw = apool.tile([P, 512], BF16, tag="raw", bufs=2, name=f"raw_{h}{p}")
            with nc.allow_low_precision(reason="softmax numerator to bf16"):
                nc.vector.tensor_copy(raw[0:64, :], av_x[0:64, :])
            out.append((raw, rb))
        return out

    def normalize(p, ch, parts):
        # out_attn.T[h] = raw[0:64] * (1/denom) broadcast; off critical path.
        # Odd head first: its partition-shifting DMA is the longest pole.
        cs = slice(ch * 512, (ch + 1) * 512)
        (raw_e, rb_e), (raw_o, rb_o) = parts
        tmp = apool.tile([P, 512], BF16, tag="tmp", bufs=2, name="tmp")
        nc.vector.tensor_mul(tmp[0:64, :], raw_o[0:64, :], rb_o[0:64, :])
        nc.sync.dma_start(oat[p][64:128, cs], tmp[0:64, :])
        nc.vector.tensor_mul(oat[p][0:64, cs], raw_e[0:64, :], rb_e[0:64, :])

    # av lags exp by AVLAG chunk-iters so a finished chunk's drain/normalize
    # chain has slack before its PSUM banks are reused.
    AVLAG = 6
    flat = [(p, ch, mt) for p in range(PAIRS) for ch in range(NCH)
            for mt in range(MT)]
    nflat = len(flat)
    st_t = {0: st_chunk(*flat[0])}
    pt_t = {}
    pending_norm = []

    def av_step(iav):
        p, ch, mt = flat[iav]
        av_chunk(p, ch, mt, pt_t.pop(iav))
        if mt == MT - 1:
            pending_norm.append((p, ch, drain_av(p, ch)))

    for i in range(nflat):
        # exp(i) first (Act consumes st(i) finished last step); fills right
        # after so their PSUM slot is copied out and released by the time the
        # trailing st(i+1) rotates into it
        pt_t[i] = exp_chunk(st_t.pop(i))
        p, ch, mt = flat[i]
        fillers(p, ch * MT + mt)
        if pending_norm and (ch * MT + mt) % 2 == 0:
            normalize(*pending_norm.pop(0))
        if i - AVLAG >= 0:
            av_step(i - AVLAG)
        if i + 1 < nflat:
            st_t[i + 1] = st_chunk(*flat[i + 1])
    for iav in range(nflat - AVLAG, nflat):
        av_step(iav)
    while pending_norm:
        normalize(*pending_norm.pop(0))  # last chunk: chain runs on DVE/Pool

    # ---- output projection + bias, two rounds of 8 groups ----
    # Round 1 accumulates ci=0..6 while the last pair's normalize completes,
    # then takes +wp[7]@oat[7]; round 2 runs all 8.
    def proj_psums():
        tiles = []
        for _ in range(3):
            big = psum.tile([P, 2, 512], FP32, tag="st", bufs=3, name="proj_ps")
            tiles.extend([big[:, 0, :], big[:, 1, :]])
        for t in ("av_e", "av_o"):
            tiles.append(psum.tile([P, 512], FP32, tag=t, bufs=1, name="proj_ps"))
        return tiles

    def proj_mm(ps, dj, ch, ci, start, stop):
        cs = slice(ch * 512, (ch + 1) * 512)
        nc.tensor.matmul(ps[:], wp[ci][:, dj * P : (dj + 1) * P],
                         oat[ci][:, cs], start=start, stop=stop)

    def proj_store(ps, dj, ch):
        cs = slice(ch * 512, (ch + 1) * 512)
        yst = ypool.tile([P, 512], BF16, tag="yst", bufs=2, name="yst")
        nc.vector.tensor_scalar_add(yst[:], ps[:], bias_sb[:, dj : dj + 1])
        nc.sync.dma_start(yt[dj * P : (dj + 1) * P, cs], yst[:])

    # The two column chunks of one dj share every wp stationary: interleave
    # their accumulation so same-weight matmuls run back-to-back and the PE
    # can skip/hide the redundant weight reloads.
    groups = [divmod(g, NCH) for g in range(CT * NCH)]  # (dj, ch)
    round1, round2 = groups[:8], groups[8:]
    tiles1 = proj_psums()
    for g in range(0, 8, 2):
        (dj, _), (tA, tB) = round1[g], tiles1[g : g + 2]
        for ci in range(CT - 1):
            proj_mm(tA, dj, 0, ci, ci == 0, False)
            proj_mm(tB, dj, 1, ci, ci == 0, False)
    for g in range(0, 8, 2):
        (dj, _), (tA, tB) = round1[g], tiles1[g : g + 2]
        proj_mm(tA, dj, 0, CT - 1, False, True)
        proj_mm(tB, dj, 1, CT - 1, False, True)
        proj_store(tA, dj, 0)
        proj_store(tB, dj, 1)
    tiles2 = proj_psums()
    for g in range(0, 8, 2):
        (dj, _), (tA, tB) = round2[g], tiles2[g : g + 2]
        for ci in range(CT):
            proj_mm(tA, dj, 0, ci, ci == 0, ci == CT - 1)
            proj_mm(tB, dj, 1, ci, ci == 0, ci == CT - 1)
        proj_store(tA, dj, 0)
        proj_store(tB, dj, 1)

    ctx.close()


def build_nc(reps=1):
    nc = bacc.Bacc("TRN2", target_bir_lowering=False, debug=False,
                   num_devices=NCORES)
    aps = {}
    aps["xt"] = nc.dram_tensor("xt", [C, N], BF16, kind="ExternalInput").ap()
    aps["wqt"] = nc.dram_tensor("wqt", [PAIRS, P, C], BF16, kind="ExternalInput").ap()
    aps["wkt"] = nc.dram_tensor("wkt", [PAIRS, P, C], BF16, kind="ExternalInput").ap()
    aps["wvt"] = nc.dram_tensor("wvt", [PAIRS // 2, P, 2 * C], BF16,
                                kind="ExternalInput").ap()
    aps["wpt"] = nc.dram_tensor("wpt", [C, C], BF16, kind="ExternalInput").ap()
    aps["bias"] = nc.dram_tensor("bias", [P, 8], FP32, kind="ExternalInput").ap()
    aps["yt"] = nc.dram_tensor("yt", [C, TOWN], BF16, kind="ExternalOutput").ap()
    with tile.TileContext(nc) as tc:
        for _ in range(reps):
            _emit(tc, aps)
    nc.compile()
    return nc


def make_in_maps(x, w_qkv, w_proj, b_proj):
    bf = ml_dtypes.bfloat16

    def part_major(w):  # [128 feats, C] -> [part, (ci, f)] flat [128, C]
        # arr[part, ci*P + f] = w[f, ci*P + part]
        return np.ascontiguousarray(
            w.T.reshape(CT, P, P).transpose(1, 0, 2).reshape(P, C)
        )

    wq_t = np.stack([part_major(w_qkv[0:C][p * P : (p + 1) * P]) for p in range(PAIRS)])
    wk_t = np.stack([part_major(w_qkv[C : 2 * C][p * P : (p + 1) * P]) for p in range(PAIRS)])

    def duo_major(d):  # [part, (ci, pp, f)] flat [128, 2C]
        wv = w_qkv[2 * C : 3 * C]
        sl = np.stack([wv[(2 * d + pp) * P : (2 * d + pp + 1) * P] for pp in range(2)])
        arr = sl.reshape(2, P, CT, P).transpose(3, 2, 0, 1)  # [part, ci, pp, f]
        return np.ascontiguousarray(arr.reshape(P, 2 * C))

    wv_t = np.stack([duo_major(d) for d in range(PAIRS // 2)])
    wp_t = np.ascontiguousarray(w_proj.T)
    bias = np.ascontiguousarray(
        np.asarray(b_proj, np.float32).reshape(8, P).T
    )
    wq_t, wk_t, wv_t, wp_t = (a.astype(bf) for a in (wq_t, wk_t, wv_t, wp_t))
    in_maps = []
    for core in range(NCORES):
        b, half = divmod(core, 2)
        xTb = np.asarray(x[b], np.float32).T  # [c, t]
        own = xTb[:, half * TOWN : (half + 1) * TOWN]
        other = xTb[:, (1 - half) * TOWN : (2 - half) * TOWN]
        # rotate so this core's q tokens are always columns 0..1023 (softmax
        # over keys is permutation-invariant, k and v use the same order)
        xt_rot = np.ascontiguousarray(np.concatenate([own, other], 1)).astype(bf)
        in_maps.append({"xt": xt_rot, "wqt": wq_t, "wkt": wk_t,
                        "wvt": wv_t, "wpt": wp_t, "bias": bias})
    return in_maps


def assemble_output(results):
    y = np.empty((B, N, C), np.float32)
    for core in range(NCORES):
        b, half = divmod(core, 2)
        y[b, half * TOWN : (half + 1) * TOWN, :] = results[core]["yt"].astype(np.float32).T
    return y


def run(x, w_qkv, w_proj, b_proj, trace=False):
    if "nc" not in _CACHE:
        _CACHE["nc"] = build_nc()
    nc = _CACHE["nc"]
    in_maps = make_in_maps(x, w_qkv, w_proj, b_proj)
    res = run_bass_kernel_spmd(nc, in_maps, list(range(NCORES)), trace=trace)
    return assemble_output(res.results), res


def kernel(x, w_qkv, w_proj, b_proj):
    y, _ = run(x, w_qkv, w_proj, b_proj)
    return y


# revision 51
# speedup vs baseline: 1.0495x; 1.0495x over previous
"""Trainium2 Bass kernel: multi-head attention block (dense transformer).

Reference computation (fp32):
    qkv = x @ w_qkv.T            x:[4,2048,1024]  w_qkv:[3072,1024]
    q,k,v per 16 heads (hd=64);  S = q@k.T * hd**-0.5; P = softmax(S)
    out = (P@v) heads-merged;    y = out @ w_proj.T + b_proj

Sharding (8 cores, no collectives): core = (batch b, token-half).  Each core
computes k/v for its whole batch (replicated across the 2 half-cores) and
q / attention / proj for its own 1024 tokens, writing a disjoint
y[b, half] slice.

All matmul operands bf16 with fp32 PSUM accumulation (fp8 would be ~2x
faster on the PE but softmax weight noise passes straight through to the
output -- the positive-sum normalization shrinks signal and noise equally --
so per-element precision must stay at bf16 for the 2e-2 gate).

On-chip layout: feature-major ([d, t]), no activation transposes:
    kT,qT: [d, t] from matmul(lhsT=w.T tile, rhs=x.T tile)
    S.T [m, (e|o) n-chunk]: per chunk-iter one [128, 2, 512] PSUM tile, two
          matmuls (two heads side by side) so ONE ScalarE Exp covers 1024
          columns.
    P.T = Exp(S.T * scale) bf16 (max-subtraction unnecessary: |S*scale|<~7)
    v_aug [t, 65] per head: v with a ones column -> attn@v matmul yields
          out.T[0:64] AND the softmax denominators in row 64, accumulated
          over m in PSUM.
    normalize (deferred): reciprocal runs straight off the PSUM denom row,
          raw numerators copy to SBUF so the banks free immediately; the
          partition-0 DMA -> GpSimd broadcast -> multiply chain runs off the
          critical path, writing oat.
    yT = matmul(lhsT=w_proj.T, rhs=out_attn.T) + bias (DVE add)

Schedule: one flat software pipeline over (pair, col-chunk, m-tile) at
512-column granularity -- exp of chunk i issues first, attn@v lags AVLAG
behind, scores for chunk i+1 issue last.  Iterating m-tiles innermost makes
the attn@v accumulators single PSUM banks, so the score tag gets THREE
[128,1024] buffers: every buffer-rotation user (scores, k/q/v projection
fillers) is >= a full step away from its buffer-mate's reader and the PE
never idles on the score->exp ping-pong.  k/q/v projections for later pairs
are woven in as PE filler work; per-pair weights stream with 2-deep
prefetch, one DMA per tensor (host pre-arranges layouts for 2KB lines).
The output projection runs in two rounds of 8 groups; round 1 accumulates
pairs 0..6 while the last chunk's normalize chain completes.
"""

import os

os.environ.setdefault("MYCRO_LOCAL_CACHE", "1")

from contextlib import ExitStack

import ml_dtypes
import numpy as np

import concourse.tile as tile
from concourse import bacc, mybir
from concourse.bass_utils import run_bass_kernel_spmd

# Problem shape (hardcoded per contract)
B, N, C = 4, 2048, 1024
HEADS, HD = 16, 64
SCALE = HD**-0.5  # 0.125
TOWN = 1024  # q tokens owned per core
NCORES = 8
P = 128
CT = C // P  # 8 contraction tiles
MT = N // P  # 16 m (key-token) tiles
PAIRS = HEADS // 2  # 8 head pairs (2 heads share a 128-row tile)
KCH = N // 512  # 4 k-token chunks of 512
NCH = TOWN // 512  # 2 q-token chunks of 512

FP32 = mybir.dt.float32
BF16 = mybir.dt.bfloat16
EXP = mybir.ActivationFunctionType.Exp

_CACHE = {}


def _emit(tc, aps):
    nc = tc.nc
    xt, wqt, wkt, wvt, wpt, bias_d, yt = (
        aps["xt"], aps["wqt"], aps["wkt"], aps["wvt"], aps["wpt"],
        aps["bias"], aps["yt"],
    )

    ctx = ExitStack()
    const_pool = ctx.enter_context(tc.tile_pool(name="const", bufs=1))
    wpool = ctx.enter_context(tc.tile_pool(name="w", bufs=1))
    xpool = ctx.enter_context(tc.tile_pool(name="x", bufs=1))
    kqv = ctx.enter_context(tc.tile_pool(name="kqv", bufs=1))
    apool = ctx.enter_context(tc.tile_pool(name="attn", bufs=1))
    opool = ctx.enter_context(tc.tile_pool(name="oattn", bufs=1))
    ypool = ctx.enter_context(tc.tile_pool(name="y", bufs=1))
    psum = ctx.enter_context(tc.tile_pool(name="ps", bufs=1, space="PSUM"))

    bias_sb = const_pool.tile([P, 8], FP32, name="bias_sb")

    # x loads + per-pair weight slices.  Host layouts give every DMA >=1KB
    # contiguous lines and one dma_start per tensor slice:
    #   wqt/wkt: [PAIRS, P, C]    (partition-major: [part, ci*P+f])
    #   wvt:     [DUOS,  P, 2C]   (partition-major: [part, (ci, pp, f)])
    wp = [wpool.tile([P, C], BF16, name=f"wp{i}", tag=f"wp{i}") for i in range(CT)]
    xs = [xpool.tile([P, N], BF16, name=f"x{i}", tag=f"x{i}") for i in range(CT)]
    wpair = {}

    def load_pair_weights(p):
        for kind, src in (("k", wkt), ("q", wqt)):
            t = wpool.tile([P, C], BF16, tag=f"w{kind}p", bufs=2,
                           name=f"w{kind}p{p}")
            wpair[(kind, p)] = t
            nc.sync.dma_start(t[:], src[p])

    def load_duo_weights(duo):
        """v weights for a duo (pairs 2*duo, 2*duo+1): [128, CT x 256] tile."""
        t = wpool.tile([P, CT, 2 * P], BF16, tag="wvd", bufs=2, name=f"wvd{duo}")
        wpair[("v", duo)] = t
        nc.sync.dma_start(t[:], wvt[duo])

    # startup loads, ordered by first use
    wk0 = wpool.tile([P, C], BF16, tag="wkp", bufs=2, name="wkp0")
    wq0 = wpool.tile([P, C], BF16, tag="wqp", bufs=2, name="wqp0")
    wpair[("k", 0)], wpair[("q", 0)] = wk0, wq0
    rows = lambda i: slice(i * P, (i + 1) * P)
    nc.sync.dma_start(wk0[:], wkt[0])
    for i in range(CT):
        nc.sync.dma_start(xs[i][:, 0:512], xt[rows(i), 0:512])
    nc.sync.dma_start(wq0[:], wqt[0])
    load_duo_weights(0)
    load_pair_weights(1)
    # bulk of x on the scalar queue so in-loop weight prefetches (sync queue)
    # aren't stuck behind these large transfers
    for i in range(CT):
        nc.scalar.dma_start(xs[i][:, 512:2048], xt[rows(i), 512:2048])
    nc.sync.dma_start(bias_sb[:], bias_d[:])

    # persistent activations
    kt = [kqv.tile([P, N], BF16, name=f"kt{p}", tag=f"kt{p}") for p in range(CT)]
    qt = [kqv.tile([P, TOWN], BF16, name=f"qt{p}", tag=f"qt{p}") for p in range(CT)]
    # v_aug per pair: [128 tokens, 16 m-tiles, 2 heads, 65] bf16; col 64 = ones
    va = [kqv.tile([P, MT, 2, HD + 1], BF16, name=f"va{p}", tag=f"va{p}")
          for p in range(PAIRS)]
    for p in range(PAIRS):
        nc.vector.memset(va[p][:, :, :, HD : HD + 1], 1.0)
    oat = [opool.tile([P, TOWN], BF16, name=f"oat{p}", tag=f"oat{p}")
           for p in range(PAIRS)]

    def kq_group(p, kind, *chs):
        """One or two 512-col chunks of the k/q projection for pair p.

        Two chunks share every weight stationary, so their accumulations are
        interleaved: same-stationary matmuls run back-to-back and the PE
        skips/hides the redundant weight reloads.
        """
        w, dst = wpair[(kind, p)], (kt if kind == "k" else qt)
        tiles = [psum.tile([P, 512], FP32, tag="st", bufs=3, name="fill_st")
                 for _ in chs]
        for ci in range(CT):
            for ps, ch in zip(tiles, chs):
                nc.tensor.matmul(
                    ps[:], w[:, ci * P : (ci + 1) * P],
                    xs[ci][:, ch * 512 : (ch + 1) * 512],
                    start=(ci == 0), stop=(ci == CT - 1),
                )
        for ps, ch in zip(tiles, chs):
            nc.vector.tensor_copy(dst[p][:, ch * 512 : (ch + 1) * 512], ps[:])

    def v_group(duo, mt):
        """v for token tile mt, one duo = 2 pairs (256 d-cols), just-in-time."""
        w = wpair[("v", duo)]
        ps = psum.tile([P, 2 * P], FP32, tag="st", bufs=3, name="fill_st")
        for ci in range(CT):
            nc.tensor.matmul(
                ps[:], xs[ci][:, mt * P : (mt + 1) * P], w[:, ci, :],
                start=(ci == 0), stop=(ci == CT - 1),
            )
        for pp in range(2):
            nc.vector.tensor_copy(
                va[2 * duo + pp][:, mt, :, 0:HD],
                ps[:, pp * P : (pp + 1) * P].rearrange("t (h d) -> t h d", h=2),
            )

    # startup: only the k/q chunks the first score steps need (k cols 0:512
    # cover m-tiles 0..3, q chunk 0 covers the first 16 chunk-steps); the
    # remaining pair-0 projection chunks are woven in as early fillers
    kq_jobs = [("k", ch) for ch in range(KCH)] + [("q", ch) for ch in range(NCH)]
    kq_group(0, "k", 0)
    kq_group(0, "q", 0)

    # ---- attention pipeline over (pair, col-chunk, m-tile) ----
    av_cur = {}

    def st_chunk(p, ch, mt):
        """Scores for one 512-col q chunk, both heads side by side so one
        ScalarE Exp covers 1024 columns."""
        st = psum.tile([P, 2, 512], FP32, tag="st", bufs=3,
                       name=f"st{p}_{ch}_{mt}")
        ms = slice(mt * P, (mt + 1) * P)
        cs = slice(ch * 512, (ch + 1) * 512)
        for h in range(2):
            nc.tensor.matmul(st[:, h, :], kt[p][64 * h : 64 * h + 64, ms],
                             qt[p][64 * h : 64 * h + 64, cs],
                             start=True, stop=True)
        return st

    def exp_chunk(st):
        pt = apool.tile([P, 2, 512], BF16, tag="pt", bufs=9, name="pt")
        nc.scalar.activation(pt[:], st[:], EXP, scale=SCALE)
        return pt

    def av_chunk(p, ch, mt, pt):
        if mt == 0:
            av_cur["e"] = psum.tile([P, 512], FP32, tag="av_e",
                                    name=f"av_e{p}_{ch}")
            av_cur["o"] = psum.tile([P, 512], FP32, tag="av_o",
                                    name=f"av_o{p}_{ch}")
        nc.tensor.matmul(av_cur["e"][0:65, :], va[p][:, mt, 0, :], pt[:, 0, :],
                         start=(mt == 0), stop=(mt == MT - 1))
        nc.tensor.matmul(av_cur["o"][0:65, :], va[p][:, mt, 1, :], pt[:, 1, :],
                         start=(mt == 0), stop=(mt == MT - 1))

    def fillers(p, vmt):
        """Filler work at virtual step vmt = ch*MT + mt within pair p."""
        if vmt == 1 and 1 <= p < PAIRS - 1:
            load_pair_weights(p + 1)
        d = p // 2
        if p == 0:
            if vmt == 1:
                load_duo_weights(1)
            # rest of pair 0's own k/q projection, just ahead of first use
            if vmt == 1:
                kq_group(0, "k", 1, 2)
            elif vmt == 8:
                kq_group(0, "k", 3)
            elif vmt == 12:
                kq_group(0, "q", 1)
            if vmt < MT:
                v_group(0, vmt)  # duo 0: one group per step, first pass
        elif p % 2 == 0:
            if vmt == 1 and d + 1 < PAIRS // 2:
                load_duo_weights(d + 1)
            if 2 <= vmt <= 16 and vmt % 2 == 0:
                v_group(d, 8 + (vmt - 2) // 2)  # tail half: tiles 8..15
        else:
            if d + 1 < PAIRS // 2 and 2 <= vmt <= 16 and vmt % 2 == 0:
                v_group(d + 1, (vmt - 2) // 2)  # head half of next duo
        if p < PAIRS - 1:
            if vmt == 17:
                kq_group(p + 1, "k", 0, 1)
            elif vmt == 21:
                kq_group(p + 1, "k", 2, 3)
            elif vmt == 25:
                kq_group(p + 1, "q", 0, 1)
        if p == 3 and vmt == 0:
            # scalar queue: these large transfers must not delay the sync
            # queue's in-loop pair-weight prefetches
            for i in range(CT):
                nc.scalar.dma_start(wp[i][:], wpt[i * P : (i + 1) * P, :])

    def drain_av(p, ch):
        """Free the av PSUM banks: reciprocal straight off the denom rows
        (starts the p0-DMA -> broadcast chain early), then raw copies."""
        out = []
        for h, av_x in (("e", av_cur["e"]), ("o", av_cur["o"])):
            r = apool.tile([P, 512], BF16, tag="rcp", bufs=2, name=f"rcp_{h}{p}")
            with nc.allow_low_precision(reason="softmax denom recip"):
                nc.vector.reciprocal(r[64:65, :], av_x[64:65, :])
            nc.sync.dma_start(r[0:1, :], r[64:65, :])
            rb = apool.tile([P, 512], BF16, tag="rb", bufs=2, name=f"rb_{h}{p}")
            nc.gpsimd.partition_broadcast(rb[0:64, :], r[0:1, :], channels=64)
            raw = apool.tile([P, 512], BF16, tag="raw", bufs=2, name=f"raw_{h}{p}")
            with nc.allow_low_precision(reason="softmax numerator to bf16"):
                nc.vector.tensor_copy(raw[0:64, :], av_x[0:64, :])
            out.append((raw, rb))
        return out

    def normalize(p, ch, parts):
        # out_attn.T[h] = raw[0:64] * (1/denom) broadcast; off critical path.
        # Odd head first: its partition-shifting DMA is the longest pole.
        cs = slice(ch * 512, (ch + 1) * 512)
        (raw_e, rb_e), (raw_o, rb_o) = parts
        tmp = apool.tile([P, 512], BF16, tag="tmp", bufs=2, name="tmp")
        nc.vector.tensor_mul(tmp[0:64, :], raw_o[0:64, :], rb_o[0:64, :])
        nc.sync.dma_start(oat[p][64:128, cs], tmp[0:64, :])
        nc.vector.tensor_mul(oat[p][0:64, cs], raw_e[0:64, :], rb_e[0:64, :])

    # av lags exp by AVLAG chunk-iters so a finished chunk's drain/normalize
    # chain has slack before its PSUM banks are reused.
    AVLAG = 6
    flat = [(p, ch, mt) for p in range(PAIRS) for ch in range(NCH)
            for mt in range(MT)]
    nflat = len(flat)
    st_t = {0: st_chunk(*flat[0])}
    pt_t = {}
    pending_norm = []

    def av_step(iav):
        p, ch, mt = flat[iav]
        av_chunk(p, ch, mt, pt_t.pop(iav))
        if mt == MT - 1:
            pending_norm.append((p, ch, drain_av(p, ch)))

    for i in range(nflat):
        # exp(i) first (Act consumes st(i) finished last step); fills right
        # after so their PSUM slot is copied out and released by the time the
        # trailing st(i+1) rotates into it
        pt_t[i] = exp_chunk(st_t.pop(i))
        p, ch, mt = flat[i]
        fillers(p, ch * MT + mt)
        if pending_norm and (ch * MT + mt) % 2 == 0:
            normalize(*pending_norm.pop(0))
        if i - AVLAG >= 0:
            av_step(i - AVLAG)
        if i + 1 < nflat:
            st_t[i + 1] = st_chunk(*flat[i + 1])
    for iav in range(nflat - AVLAG, nflat):
        av_step(iav)
    while pending_norm:
        normalize(*pending_norm.pop(0))  # last chunk: chain runs on DVE/Pool

    # ---- output projection + bias, two rounds of 8 groups ----
    # Round 1 accumulates ci=0..6 while the last pair's normalize completes,
    # then takes +wp[7]@oat[7]; round 2 runs all 8.
    def proj_psums():
        tiles = []
        for _ in range(3):
            big = psum.tile([P, 2, 512], FP32, tag="st", bufs=3, name="proj_ps")
            tiles.extend([big[:, 0, :], big[:, 1, :]])
        for t in ("av_e", "av_o"):
            tiles.append(psum.tile([P, 512], FP32, tag=t, bufs=1, name="proj_ps"))
        return tiles

    def proj_mm(ps, dj, ch, ci, start, stop):
        cs = slice(ch * 512, (ch + 1) * 512)
        nc.tensor.matmul(ps[:], wp[ci][:, dj * P : (dj + 1) * P],
                         oat[ci][:, cs], start=start, stop=stop)

    def proj_store(ps, dj, ch):
        cs = slice(ch * 512, (ch + 1) * 512)
        yst = ypool.tile([P, 512], BF16, tag="yst", bufs=2, name="yst")
        nc.vector.tensor_scalar_add(yst[:], ps[:], bias_sb[:, dj : dj + 1])
        nc.sync.dma_start(yt[dj * P : (dj + 1) * P, cs], yst[:])

    # The two column chunks of one dj share every wp stationary: interleave
    # their accumulation so same-weight matmuls run back-to-back and the PE
    # can skip/hide the redundant weight reloads.
    groups = [divmod(g, NCH) for g in range(CT * NCH)]  # (dj, ch)
    round1, round2 = groups[:8], groups[8:]
    tiles1 = proj_psums()
    for g in range(0, 8, 2):
        (dj, _), (tA, tB) = round1[g], tiles1[g : g + 2]
        for ci in range(CT - 1):
            proj_mm(tA, dj, 0, ci, ci == 0, False)
            proj_mm(tB, dj, 1, ci, ci == 0, False)
    for g in range(0, 8, 2):
        (dj, _), (tA, tB) = round1[g], tiles1[g : g + 2]
        proj_mm(tA, dj, 0, CT - 1, False, True)
        proj_mm(tB, dj, 1, CT - 1, False, True)
        proj_store(tA, dj, 0)
        proj_store(tB, dj, 1)
    tiles2 = proj_psums()
    for g in range(0, 8, 2):
        (dj, _), (tA, tB) = round2[g], tiles2[g : g + 2]
        for ci in range(CT):
            proj_mm(tA, dj, 0, ci, ci == 0, ci == CT - 1)
            proj_mm(tB, dj, 1, ci, ci == 0, ci == CT - 1)
        proj_store(tA, dj, 0)
        proj_store(tB, dj, 1)

    ctx.close()


def build_nc(reps=1):
    nc = bacc.Bacc("TRN2", target_bir_lowering=False, debug=False,
                   num_devices=NCORES)
    aps = {}
    aps["xt"] = nc.dram_tensor("xt", [C, N], BF16, kind="ExternalInput").ap()
    aps["wqt"] = nc.dram_tensor("wqt", [PAIRS, P, C], BF16, kind="ExternalInput").ap()
    aps["wkt"] = nc.dram_tensor("wkt", [PAIRS, P, C], BF16, kind="ExternalInput").ap()
    aps["wvt"] = nc.dram_tensor("wvt", [PAIRS // 2, P, 2 * C], BF16,
                                kind="ExternalInput").ap()
    aps["wpt"] = nc.dram_tensor("wpt", [C, C], BF16, kind="ExternalInput").ap()
    aps["bias"] = nc.dram_tensor("bias", [P, 8], FP32, kind="ExternalInput").ap()
    aps["yt"] = nc.dram_tensor("yt", [C, TOWN], BF16, kind="ExternalOutput").ap()
    with tile.TileContext(nc) as tc:
        for _ in range(reps):
            _emit(tc, aps)
    nc.compile()
    return nc


def make_in_maps(x, w_qkv, w_proj, b_proj):
    bf = ml_dtypes.bfloat16

    def part_major(w):  # [128 feats, C] -> [part, (ci, f)] flat [128, C]
        # arr[part, ci*P + f] = w[f, ci*P + part]
        return np.ascontiguousarray(
            w.T.reshape(CT, P, P).transpose(1, 0, 2).reshape(P, C)
        )

    wq_t = np.stack([part_major(w_qkv[0:C][p * P : (p + 1) * P]) for p in range(PAIRS)])
    wk_t = np.stack([part_major(w_qkv[C : 2 * C][p * P : (p + 1) * P]) for p in range(PAIRS)])

    def duo_major(d):  # [part, (ci, pp, f)] flat [128, 2C]
        wv = w_qkv[2 * C : 3 * C]
        sl = np.stack([wv[(2 * d + pp) * P : (2 * d + pp + 1) * P] for pp in range(2)])
        arr = sl.reshape(2, P, CT, P).transpose(3, 2, 0, 1)  # [part, ci, pp, f]
        return np.ascontiguousarray(arr.reshape(P, 2 * C))

    wv_t = np.stack([duo_major(d) for d in range(PAIRS // 2)])
    wp_t = np.ascontiguousarray(w_proj.T)
    bias = np.ascontiguousarray(
        np.asarray(b_proj, np.float32).reshape(8, P).T
    )
    wq_t, wk_t, wv_t, wp_t = (a.astype(bf) for a in (wq_t, wk_t, wv_t, wp_t))
    in_maps = []
    for core in range(NCORES):
        b, half = divmod(core, 2)
        xTb = np.asarray(x[b], np.float32).T  # [c, t]
        own = xTb[:, half * TOWN : (half + 1) * TOWN]
        other = xTb[:, (1 - half) * TOWN : (2 - half) * TOWN]
        # rotate so this core's q tokens are always columns 0..1023 (softmax
        # over keys is permutation-invariant, k and v use the same order)
        xt_rot = np.ascontiguousarray(np.concatenate([own, other], 1)).astype(bf)
        in_maps.append({"xt": xt_rot, "wqt": wq_t, "wkt": wk_t,
                        "wvt": wv_t, "wpt": wp_t, "bias": bias})
    return in_maps


def assemble_output(results):
    y = np.empty((B, N, C), np.float32)
    for core in range(NCORES):
        b, half = divmod(core, 2)
        y[b, half * TOWN : (half + 1) * TOWN, :] = results[core]["yt"].astype(np.float32).T
    return y


def run(x, w_qkv, w_proj, b_proj, trace=False):
    if "nc" not in _CACHE:
        _CACHE["nc"] = build_nc()
    nc = _CACHE["nc"]
    in_maps = make_in_maps(x, w_qkv, w_proj, b_proj)
    res = run_bass_kernel_spmd(nc, in_maps, list(range(NCORES)), trace=trace)
    return assemble_output(res.results), res


def kernel(x, w_qkv, w_proj, b_proj):
    y, _ = run(x, w_qkv, w_proj, b_proj)
    return y


# revision 52
# speedup vs baseline: 1.2860x; 1.2253x over previous
"""Trainium2 Bass kernel: multi-head attention block (dense transformer).

Reference computation (fp32):
    qkv = x @ w_qkv.T            x:[4,2048,1024]  w_qkv:[3072,1024]
    q,k,v per 16 heads (hd=64);  S = q@k.T * hd**-0.5; P = softmax(S)
    out = (P@v) heads-merged;    y = out @ w_proj.T + b_proj

Sharding (8 cores, no collectives): core = (batch b, token-half).  Each core
computes k/v for its whole batch (replicated across the 2 half-cores) and
q / attention / proj for its own 1024 tokens, writing a disjoint
y[b, half] slice.

All matmul operands bf16 with fp32 PSUM accumulation (fp8 would be ~2x
faster on the PE but softmax weight noise passes straight through to the
output -- the positive-sum normalization shrinks signal and noise equally --
so per-element precision must stay at bf16 for the 2e-2 gate).

On-chip layout: feature-major ([d, t]), no activation transposes:
    kT,qT: [d, t] from matmul(lhsT=w.T tile, rhs=x.T tile)
    S.T [m, (e|o) n-chunk]: per chunk-iter one [128, 2, 512] PSUM tile, two
          matmuls (two heads side by side) so ONE ScalarE Exp covers 1024
          columns.
    P.T = Exp(S.T * scale) bf16 (max-subtraction unnecessary: |S*scale|<~7)
    v_aug [t, 65] per head: v with a ones column -> attn@v matmul yields
          out.T[0:64] AND the softmax denominators in row 64, accumulated
          over m in PSUM.
    normalize (deferred): reciprocal runs straight off the PSUM denom row,
          raw numerators copy to SBUF so the banks free immediately; the
          partition-0 DMA -> GpSimd broadcast -> multiply chain runs off the
          critical path, writing oat.
    yT = matmul(lhsT=w_proj.T, rhs=out_attn.T) + bias (DVE add)

Schedule: one flat software pipeline over (pair, col-chunk, m-tile) at
512-column granularity -- exp of chunk i issues first, attn@v lags AVLAG
behind, scores for chunk i+1 issue last.  Iterating m-tiles innermost makes
the attn@v accumulators single PSUM banks, so the score tag gets THREE
[128,1024] buffers: every buffer-rotation user (scores, k/q/v projection
fillers) is >= a full step away from its buffer-mate's reader and the PE
never idles on the score->exp ping-pong.  k/q/v projections for later pairs
are woven in as PE filler work; per-pair weights stream with 2-deep
prefetch, one DMA per tensor (host pre-arranges layouts for 2KB lines).
The output projection runs in two rounds of 8 groups; round 1 accumulates
pairs 0..6 while the last chunk's normalize chain completes.
"""

import os

os.environ.setdefault("MYCRO_LOCAL_CACHE", "1")

from contextlib import ExitStack

import ml_dtypes
import numpy as np

import concourse.tile as tile
from concourse import bacc, mybir
from concourse.bass_utils import run_bass_kernel_spmd

# Problem shape (hardcoded per contract)
B, N, C = 4, 2048, 1024
HEADS, HD = 16, 64
SCALE = HD**-0.5  # 0.125
TOWN = 1024  # q tokens owned per core
NCORES = 8
P = 128
CT = C // P  # 8 contraction tiles
MT = N // P  # 16 m (key-token) tiles
PAIRS = HEADS // 2  # 8 head pairs (2 heads share a 128-row tile)
KCH = N // 512  # 4 k-token chunks of 512
NCH = TOWN // 512  # 2 q-token chunks of 512

FP32 = mybir.dt.float32
BF16 = mybir.dt.bfloat16
EXP = mybir.ActivationFunctionType.Exp

_CACHE = {}


def _emit(tc, aps):
    nc = tc.nc
    xt, wqt, wkt, wvt, wpt, bias_d, yt = (
        aps["xt"], aps["wqt"], aps["wkt"], aps["wvt"], aps["wpt"],
        aps["bias"], aps["yt"],
    )

    ctx = ExitStack()
    const_pool = ctx.enter_context(tc.tile_pool(name="const", bufs=1))
    wpool = ctx.enter_context(tc.tile_pool(name="w", bufs=1))
    xpool = ctx.enter_context(tc.tile_pool(name="x", bufs=1))
    kqv = ctx.enter_context(tc.tile_pool(name="kqv", bufs=1))
    apool = ctx.enter_context(tc.tile_pool(name="attn", bufs=1))
    opool = ctx.enter_context(tc.tile_pool(name="oattn", bufs=1))
    ypool = ctx.enter_context(tc.tile_pool(name="y", bufs=1))
    psum = ctx.enter_context(tc.tile_pool(name="ps", bufs=1, space="PSUM"))

    bias_sb = const_pool.tile([P, 8], FP32, name="bias_sb")

    # x loads + per-pair weight slices.  Host layouts give every DMA >=1KB
    # contiguous lines and one dma_start per tensor slice:
    #   wqt/wkt: [PAIRS, P, C]    (partition-major: [part, ci*P+f])
    #   wvt:     [DUOS,  P, 2C]   (partition-major: [part, (ci, pp, f)])
    wp = [wpool.tile([P, C], BF16, name=f"wp{i}", tag=f"wp{i}") for i in range(CT)]
    xs = [xpool.tile([P, N], BF16, name=f"x{i}", tag=f"x{i}") for i in range(CT)]
    wpair = {}

    def load_pair_weights(p):
        for kind, src in (("k", wkt), ("q", wqt)):
            t = wpool.tile([P, C], BF16, tag=f"w{kind}p", bufs=2,
                           name=f"w{kind}p{p}")
            wpair[(kind, p)] = t
            nc.sync.dma_start(t[:], src[p])

    def load_duo_weights(duo):
        """v weights for a duo (pairs 2*duo, 2*duo+1): [128, CT x 256] tile."""
        t = wpool.tile([P, CT, 2 * P], BF16, tag="wvd", bufs=2, name=f"wvd{duo}")
        wpair[("v", duo)] = t
        nc.sync.dma_start(t[:], wvt[duo])

    # startup loads, ordered by first use
    wk0 = wpool.tile([P, C], BF16, tag="wkp", bufs=2, name="wkp0")
    wq0 = wpool.tile([P, C], BF16, tag="wqp", bufs=2, name="wqp0")
    wpair[("k", 0)], wpair[("q", 0)] = wk0, wq0
    rows = lambda i: slice(i * P, (i + 1) * P)
    nc.sync.dma_start(wk0[:], wkt[0])
    for i in range(CT):
        nc.sync.dma_start(xs[i][:, 0:512], xt[rows(i), 0:512])
    nc.sync.dma_start(wq0[:], wqt[0])
    load_duo_weights(0)
    load_pair_weights(1)
    # bulk of x on the scalar queue so in-loop weight prefetches (sync queue)
    # aren't stuck behind these large transfers
    for i in range(CT):
        nc.scalar.dma_start(xs[i][:, 512:2048], xt[rows(i), 512:2048])
    nc.sync.dma_start(bias_sb[:], bias_d[:])

    # persistent activations
    kt = [kqv.tile([P, N], BF16, name=f"kt{p}", tag=f"kt{p}") for p in range(CT)]
    qt = [kqv.tile([P, TOWN], BF16, name=f"qt{p}", tag=f"qt{p}") for p in range(CT)]
    # v_aug per pair: [128 tokens, 16 m-tiles, 2 heads, 65] bf16; col 64 = ones
    va = [kqv.tile([P, MT, 2, HD + 1], BF16, name=f"va{p}", tag=f"va{p}")
          for p in range(PAIRS)]
    for p in range(PAIRS):
        nc.vector.memset(va[p][:, :, :, HD : HD + 1], 1.0)
    oat = [opool.tile([P, TOWN], BF16, name=f"oat{p}", tag=f"oat{p}")
           for p in range(PAIRS)]

    def kq_group(p, kind, ch):
        """One 512-col chunk of the k or q projection for pair p."""
        w, dst = wpair[(kind, p)], (kt if kind == "k" else qt)
        ps = psum.tile([P, 512], FP32, tag="st", bufs=3, name="fill_st")
        cols = slice(ch * 512, (ch + 1) * 512)
        for ci in range(CT):
            nc.tensor.matmul(
                ps[:], w[:, ci * P : (ci + 1) * P], xs[ci][:, cols],
                start=(ci == 0), stop=(ci == CT - 1),
            )
        nc.vector.tensor_copy(dst[p][:, cols], ps[:])

    def v_group(duo, mt):
        """v for token tile mt, one duo = 2 pairs (256 d-cols), just-in-time."""
        w = wpair[("v", duo)]
        ps = psum.tile([P, 2 * P], FP32, tag="st", bufs=3, name="fill_st")
        for ci in range(CT):
            nc.tensor.matmul(
                ps[:], xs[ci][:, mt * P : (mt + 1) * P], w[:, ci, :],
                start=(ci == 0), stop=(ci == CT - 1),
            )
        for pp in range(2):
            nc.vector.tensor_copy(
                va[2 * duo + pp][:, mt, :, 0:HD],
                ps[:, pp * P : (pp + 1) * P].rearrange("t (h d) -> t h d", h=2),
            )

    # startup: only the k/q chunks the first score steps need (k cols 0:512
    # cover m-tiles 0..3, q chunk 0 covers the first 16 chunk-steps); the
    # remaining pair-0 projection chunks are woven in as early fillers
    kq_jobs = [("k", ch) for ch in range(KCH)] + [("q", ch) for ch in range(NCH)]
    kq_group(0, "k", 0)
    kq_group(0, "q", 0)

    # ---- attention pipeline over (pair, col-chunk, m-tile) ----
    av_cur = {}

    def st_chunk(p, ch, mt):
        """Scores for one 512-col q chunk, both heads side by side so one
        ScalarE Exp covers 1024 columns."""
        st = psum.tile([P, 2, 512], FP32, tag="st", bufs=3,
                       name=f"st{p}_{ch}_{mt}")
        ms = slice(mt * P, (mt + 1) * P)
        cs = slice(ch * 512, (ch + 1) * 512)
        for h in range(2):
            nc.tensor.matmul(st[:, h, :], kt[p][64 * h : 64 * h + 64, ms],
                             qt[p][64 * h : 64 * h + 64, cs],
                             start=True, stop=True)
        return st

    def exp_chunk(st):
        pt = apool.tile([P, 2, 512], BF16, tag="pt", bufs=9, name="pt")
        nc.scalar.activation(pt[:], st[:], EXP, scale=SCALE)
        return pt

    def av_chunk(p, ch, mt, pt):
        if mt == 0:
            av_cur["e"] = psum.tile([P, 512], FP32, tag="av_e",
                                    name=f"av_e{p}_{ch}")
            av_cur["o"] = psum.tile([P, 512], FP32, tag="av_o",
                                    name=f"av_o{p}_{ch}")
        nc.tensor.matmul(av_cur["e"][0:65, :], va[p][:, mt, 0, :], pt[:, 0, :],
                         start=(mt == 0), stop=(mt == MT - 1))
        nc.tensor.matmul(av_cur["o"][0:65, :], va[p][:, mt, 1, :], pt[:, 1, :],
                         start=(mt == 0), stop=(mt == MT - 1))

    def fillers(p, vmt):
        """Filler work at virtual step vmt = ch*MT + mt within pair p."""
        if vmt == 1 and 1 <= p < PAIRS - 1:
            load_pair_weights(p + 1)
        d = p // 2
        if p == 0:
            if vmt == 1:
                load_duo_weights(1)
            # rest of pair 0's own k/q projection, just ahead of first use
            if vmt in (1, 4, 8):
                kq_group(0, "k", 1 + (1, 4, 8).index(vmt))
            elif vmt == 12:
                kq_group(0, "q", 1)
            if vmt < MT:
                v_group(0, vmt)  # duo 0: one group per step, first pass
        elif p % 2 == 0:
            if vmt == 1 and d + 1 < PAIRS // 2:
                load_duo_weights(d + 1)
            if 2 <= vmt <= 16 and vmt % 2 == 0:
                v_group(d, 8 + (vmt - 2) // 2)  # tail half: tiles 8..15
        else:
            if d + 1 < PAIRS // 2 and 2 <= vmt <= 16 and vmt % 2 == 0:
                v_group(d + 1, (vmt - 2) // 2)  # head half of next duo
        if p < PAIRS - 1 and 17 <= vmt <= 27 and vmt % 2 == 1:
            kq_group(p + 1, *kq_jobs[(vmt - 17) // 2])
        if p == 3 and vmt == 0:
            # scalar queue: these large transfers must not delay the sync
            # queue's in-loop pair-weight prefetches
            for i in range(CT):
                nc.scalar.dma_start(wp[i][:], wpt[i * P : (i + 1) * P, :])

    def drain_av(p, ch):
        """Free the av PSUM banks: reciprocal straight off the denom rows
        (starts the p0-DMA -> broadcast chain early), then raw copies."""
        out = []
        for h, av_x in (("e", av_cur["e"]), ("o", av_cur["o"])):
            r = apool.tile([P, 512], BF16, tag="rcp", bufs=2, name=f"rcp_{h}{p}")
            with nc.allow_low_precision(reason="softmax denom recip"):
                nc.vector.reciprocal(r[64:65, :], av_x[64:65, :])
            nc.sync.dma_start(r[0:1, :], r[64:65, :])
            rb = apool.tile([P, 512], BF16, tag="rb", bufs=2, name=f"rb_{h}{p}")
            nc.gpsimd.partition_broadcast(rb[0:64, :], r[0:1, :], channels=64)
            raw = apool.tile([P, 512], BF16, tag="raw", bufs=2, name=f"raw_{h}{p}")
            with nc.allow_low_precision(reason="softmax numerator to bf16"):
                nc.vector.tensor_copy(raw[0:64, :], av_x[0:64, :])
            out.append((raw, rb))
        return out

    def normalize(p, ch, parts):
        # out_attn.T[h] = raw[0:64] * (1/denom) broadcast; off critical path.
        # Odd head first: its partition-shifting DMA is the longest pole.
        cs = slice(ch * 512, (ch + 1) * 512)
        (raw_e, rb_e), (raw_o, rb_o) = parts
        tmp = apool.tile([P, 512], BF16, tag="tmp", bufs=2, name="tmp")
        nc.vector.tensor_mul(tmp[0:64, :], raw_o[0:64, :], rb_o[0:64, :])
        nc.sync.dma_start(oat[p][64:128, cs], tmp[0:64, :])
        nc.vector.tensor_mul(oat[p][0:64, cs], raw_e[0:64, :], rb_e[0:64, :])

    # av lags exp by AVLAG chunk-iters so a finished chunk's drain/normalize
    # chain has slack before its PSUM banks are reused.
    AVLAG = 6
    flat = [(p, ch, mt) for p in range(PAIRS) for ch in range(NCH)
            for mt in range(MT)]
    nflat = len(flat)
    st_t = {0: st_chunk(*flat[0])}
    pt_t = {}
    pending_norm = []

    def av_step(iav):
        p, ch, mt = flat[iav]
        av_chunk(p, ch, mt, pt_t.pop(iav))
        if mt == MT - 1:
            pending_norm.append((p, ch, drain_av(p, ch)))

    for i in range(nflat):
        # exp(i) first (Act consumes st(i) finished last step); fills right
        # after so their PSUM slot is copied out and released by the time the
        # trailing st(i+1) rotates into it
        pt_t[i] = exp_chunk(st_t.pop(i))
        p, ch, mt = flat[i]
        fillers(p, ch * MT + mt)
        if pending_norm and (ch * MT + mt) % 2 == 0:
            normalize(*pending_norm.pop(0))
        if i - AVLAG >= 0:
            av_step(i - AVLAG)
        if i + 1 < nflat:
            st_t[i + 1] = st_chunk(*flat[i + 1])
    for iav in range(nflat - AVLAG, nflat):
        av_step(iav)
    while pending_norm:
        normalize(*pending_norm.pop(0))  # last chunk: chain runs on DVE/Pool

    # ---- output projection + bias, two rounds of 8 groups ----
    # Round 1 accumulates ci=0..6 while the last pair's normalize completes,
    # then takes +wp[7]@oat[7]; round 2 runs all 8.
    def proj_psums():
        tiles = []
        for _ in range(3):
            big = psum.tile([P, 2, 512], FP32, tag="st", bufs=3, name="proj_ps")
            tiles.extend([big[:, 0, :], big[:, 1, :]])
        for t in ("av_e", "av_o"):
            tiles.append(psum.tile([P, 512], FP32, tag=t, bufs=1, name="proj_ps"))
        return tiles

    def proj_mm(ps, dj, ch, ci, start, stop):
        cs = slice(ch * 512, (ch + 1) * 512)
        nc.tensor.matmul(ps[:], wp[ci][:, dj * P : (dj + 1) * P],
                         oat[ci][:, cs], start=start, stop=stop)

    def proj_store(ps, dj, ch):
        cs = slice(ch * 512, (ch + 1) * 512)
        yst = ypool.tile([P, 512], BF16, tag="yst", bufs=2, name="yst")
        nc.vector.tensor_scalar_add(yst[:], ps[:], bias_sb[:, dj : dj + 1])
        nc.sync.dma_start(yt[dj * P : (dj + 1) * P, cs], yst[:])

    # The two column chunks of one dj share every wp stationary: interleave
    # their accumulation so same-weight matmuls run back-to-back and the PE
    # can skip/hide the redundant weight reloads.
    groups = [divmod(g, NCH) for g in range(CT * NCH)]  # (dj, ch)
    round1, round2 = groups[:8], groups[8:]
    tiles1 = proj_psums()
    for g in range(0, 8, 2):
        (dj, _), (tA, tB) = round1[g], tiles1[g : g + 2]
        for ci in range(CT - 1):
            proj_mm(tA, dj, 0, ci, ci == 0, False)
            proj_mm(tB, dj, 1, ci, ci == 0, False)
    for g in range(0, 8, 2):
        (dj, _), (tA, tB) = round1[g], tiles1[g : g + 2]
        proj_mm(tA, dj, 0, CT - 1, False, True)
        proj_mm(tB, dj, 1, CT - 1, False, True)
        proj_store(tA, dj, 0)
        proj_store(tB, dj, 1)
    tiles2 = proj_psums()
    for g in range(0, 8, 2):
        (dj, _), (tA, tB) = round2[g], tiles2[g : g + 2]
        for ci in range(CT):
            proj_mm(tA, dj, 0, ci, ci == 0, ci == CT - 1)
            proj_mm(tB, dj, 1, ci, ci == 0, ci == CT - 1)
        proj_store(tA, dj, 0)
        proj_store(tB, dj, 1)

    ctx.close()


def build_nc(reps=1):
    nc = bacc.Bacc("TRN2", target_bir_lowering=False, debug=False,
                   num_devices=NCORES)
    aps = {}
    aps["xt"] = nc.dram_tensor("xt", [C, N], BF16, kind="ExternalInput").ap()
    aps["wqt"] = nc.dram_tensor("wqt", [PAIRS, P, C], BF16, kind="ExternalInput").ap()
    aps["wkt"] = nc.dram_tensor("wkt", [PAIRS, P, C], BF16, kind="ExternalInput").ap()
    aps["wvt"] = nc.dram_tensor("wvt", [PAIRS // 2, P, 2 * C], BF16,
                                kind="ExternalInput").ap()
    aps["wpt"] = nc.dram_tensor("wpt", [C, C], BF16, kind="ExternalInput").ap()
    aps["bias"] = nc.dram_tensor("bias", [P, 8], FP32, kind="ExternalInput").ap()
    aps["yt"] = nc.dram_tensor("yt", [C, TOWN], BF16, kind="ExternalOutput").ap()
    with tile.TileContext(nc) as tc:
        for _ in range(reps):
            _emit(tc, aps)
    nc.compile()
    return nc


def make_in_maps(x, w_qkv, w_proj, b_proj):
    bf = ml_dtypes.bfloat16

    def part_major(w):  # [128 feats, C] -> [part, (ci, f)] flat [128, C]
        # arr[part, ci*P + f] = w[f, ci*P + part]
        return np.ascontiguousarray(
            w.T.reshape(CT, P, P).transpose(1, 0, 2).reshape(P, C)
        )

    wq_t = np.stack([part_major(w_qkv[0:C][p * P : (p + 1) * P]) for p in range(PAIRS)])
    wk_t = np.stack([part_major(w_qkv[C : 2 * C][p * P : (p + 1) * P]) for p in range(PAIRS)])

    def duo_major(d):  # [part, (ci, pp, f)] flat [128, 2C]
        wv = w_qkv[2 * C : 3 * C]
        sl = np.stack([wv[(2 * d + pp) * P : (2 * d + pp + 1) * P] for pp in range(2)])
        arr = sl.reshape(2, P, CT, P).transpose(3, 2, 0, 1)  # [part, ci, pp, f]
        return np.ascontiguousarray(arr.reshape(P, 2 * C))

    wv_t = np.stack([duo_major(d) for d in range(PAIRS // 2)])
    wp_t = np.ascontiguousarray(w_proj.T)
    bias = np.ascontiguousarray(
        np.asarray(b_proj, np.float32).reshape(8, P).T
    )
    wq_t, wk_t, wv_t, wp_t = (a.astype(bf) for a in (wq_t, wk_t, wv_t, wp_t))
    in_maps = []
    for core in range(NCORES):
        b, half = divmod(core, 2)
        xTb = np.asarray(x[b], np.float32).T  # [c, t]
        own = xTb[:, half * TOWN : (half + 1) * TOWN]
        other = xTb[:, (1 - half) * TOWN : (2 - half) * TOWN]
        # rotate so this core's q tokens are always columns 0..1023 (softmax
        # over keys is permutation-invariant, k and v use the same order)
        xt_rot = np.ascontiguousarray(np.concatenate([own, other], 1)).astype(bf)
        in_maps.append({"xt": xt_rot, "wqt": wq_t, "wkt": wk_t,
                        "wvt": wv_t, "wpt": wp_t, "bias": bias})
    return in_maps


def assemble_output(results):
    y = np.empty((B, N, C), np.float32)
    for core in range(NCORES):
        b, half = divmod(core, 2)
        y[b, half * TOWN : (half + 1) * TOWN, :] = results[core]["yt"].astype(np.float32).T
    return y


def run(x, w_qkv, w_proj, b_proj, trace=False):
    if "nc" not in _CACHE:
        _CACHE["nc"] = build_nc()
    nc = _CACHE["nc"]
    in_maps = make_in_maps(x, w_qkv, w_proj, b_proj)
    res = run_bass_kernel_spmd(nc, in_maps, list(range(NCORES)), trace=trace)
    return assemble_output(res.results), res


def kernel(x, w_qkv, w_proj, b_proj):
    y, _ = run(x, w_qkv, w_proj, b_proj)
    return y
